# revision 12
# baseline (speedup 1.0000x reference)
"""Trainium2 Bass kernel for CrossGraphAttention (ragged per-graph MHA + linear).

The grading cost is dominated by host<->device transfer through the axon
PJRT relay (~25-130 MB/s, drifting; device compute is ~1.5 ms/call), so the
design minimizes bytes on the wire per call and overlaps transfer
directions:

  * int8 per-row quantized I/O: x is quantized host-side to int8 with a
    per-token fp16 scale (rel-err contribution ~8e-3, tolerance 2e-2); y is
    quantized device-side to int8 with a per-token fp16 scale (the HW
    converts float->int8 with round-to-nearest; validated by probe). 8.4 MB
    up + 8.4 MB down total instead of 2x 16.8 MB fp16 and 17 MB of donated
    zero-output upload in the stock path (52 MB -> 17 MB per call; measured
    4.1x faster than the previous kernel in an interleaved A/B).
  * weights/masks/identity live in small per-wave blobs uploaded ONCE and
    cached as committed device arrays across kernel() calls; the jit
    executable and NEFF are cached too, so warm calls skip retrace.
  * custom mirror of bass2jax.run_bass_via_pjrt without output-buffer
    donation: PJRT-allocated uninitialized outputs are fine because the
    host only reads rows the kernel wrote.
  * the call is split into WAVES pipelined executions of one shared NEFF
    (graphs are grouped into N_CORES*WAVES lanes; slot s of every lane is
    padded to the length of the rank-(LANES*s) graph so all lanes share one
    layout). Wave k+1's host quantize + upload overlap wave k's execute +
    download (the relay carries H2D and D2H concurrently at ~1.5x the
    serial rate); fetches run in background threads.
  * x ships in natural [token, 512] int8 layout (host does one vectorized
    quantize pass + contiguous row copies); the device converts to fp16,
    transposes via PE identity matmuls, and de-quantizes columns with a
    rank-1 broadcast of the scale row, producing the x^T layout the QKV
    projection needs. No strided host-side transposes in the per-call path.

Final config: 1 core x 4 waves (the relay is a single ~40 MB/s shared
pipe under current conditions -- extra cores only add dispatch overhead;
4 waves keep upload/execute/download fully overlapped). Measured: device
exec 1.15 ms/wave (reps-marginal), ~82 ms zero-payload dispatch RTT per
call (fully hidden by the wave pipeline), warm call within ~2% of the
bytes/bandwidth model -- the kernel is transport-floor-bound.

Device program per core/wave (data-parallel over graphs):
  1. load+transpose+dequant x into SBUF-resident x^T (per graph).
  2. QKV projection from SBUF x^T (q^T, k^T per head row-tiles; V natural).
  3. attention: scores TRANSPOSED (S^T[k,q]) per head-pair, exp fused with
     PSUM->SBUF eviction (scalar engine) with key-padding mask via a
     per-partition bias (-60000 -> exp 0; valid tokens get -8 so fp16 P
     stays in range, cancels in softmax); denominator via ones-matmul;
     ctx^T accumulated over k-tiles in PSUM; normalization by 1/denom via
     rank-1 broadcast matmul + vector multiply.
  4. fused output projection y = ctx @ (lin_w @ out_proj_w)^T, then per-row
     absmax -> int8 quantize + fp16 scale, DMA out.
"""

import threading

import numpy as np

import concourse.mybir as mybir
import concourse.tile as tile
from concourse import bacc, bass2jax

import jax
from jax.sharding import Mesh, PartitionSpec, NamedSharding
from jax.experimental.shard_map import shard_map

F32 = mybir.dt.float32
F16 = mybir.dt.float16
I8 = mybir.dt.int8

NG = 16          # number of graphs
N_CORES = 1      # cores used per wave (relay is one ~40MB/s pipe; more cores
                 # only add dispatch overhead)
WAVES = 4        # pipelined executions per call
LANES = N_CORES * WAVES
GPL = NG // LANES     # graphs (slots) per lane
E = 512
H = 8
D = 64
NEG = -60000.0   # exp(scale*s + NEG) == 0; representable in fp16

_cache = {}


def _qb_splits(n):
    out = [512] * (n // 512)
    if n % 512:
        out.append(n % 512)
    return out


def _layout(lengths):
    """lengths: per-slot padded graph lengths (shared by all lanes).
    Returns slot offsets + layout of the per-lane weight blob [128, WCOL]."""
    T_pad = sum(lengths)
    soffs = [0]
    for L in lengths[:-1]:
        soffs.append(soffs[-1] + L)
    kts = [(L + 127) // 128 for L in lengths]
    moffs = [0]
    for k in kts[:-1]:
        moffs.append(moffs[-1] + k)
    woff = 0                       # 4 chunks of 1536 cols (rows 128e, Wqkv^T)
    poff = woff + 4 * 3 * E        # 4 chunks of 512 cols (rows 128e, Wp^T)
    moff = poff + 4 * E            # sum(kts) cols of per-partition mask bias
    ioff = moff + sum(kts)         # 128 cols fp16 identity
    wcol = ioff + 128
    return T_pad, soffs, kts, moffs, woff, poff, moff, ioff, wcol


def _build(lengths, reps=1):
    """Build + compile the Bass program for per-slot graph lengths."""
    T_pad, SOFFS, KTS, MOFFS, WOFF, POFF, MOFF, IOFF, WCOL = _layout(lengths)
    L0 = max(lengths)
    KT0 = (L0 + 127) // 128

    nc = bacc.Bacc("TRN2", target_bir_lowering=False, debug=False,
                   enable_asserts=False)

    xq_d = nc.dram_tensor("xq", [T_pad, E], I8, kind="ExternalInput")
    xs_d = nc.dram_tensor("xs", [1, T_pad], F16, kind="ExternalInput")
    wb_d = nc.dram_tensor("wb", [128, WCOL], F16, kind="ExternalInput")
    yq_d = nc.dram_tensor("yq", [T_pad, E], I8, kind="ExternalOutput")
    ys_d = nc.dram_tensor("ys", [T_pad, 1], F16, kind="ExternalOutput")

    with tile.TileContext(nc) as tc:
        with (
            tc.tile_pool(name="const", bufs=1) as cpool,
            tc.tile_pool(name="xt", bufs=2) as xtpool,
            tc.tile_pool(name="xin", bufs=3) as xinpool,
            tc.tile_pool(name="qkv", bufs=2) as qkvpool,
            tc.tile_pool(name="pt", bufs=4) as ptpool,
            tc.tile_pool(name="small", bufs=3) as smallpool,
            tc.tile_pool(name="ctxn", bufs=3) as ctxnpool,
            tc.tile_pool(name="yout", bufs=3) as ypool,
            tc.tile_pool(name="spsum", bufs=2, space="PSUM") as spsum,
            tc.tile_pool(name="cpsum", bufs=2, space="PSUM") as cpsum,
            tc.tile_pool(name="mpsum", bufs=2, space="PSUM") as mpsum,
        ):
            # ---- constants / weights (resident) ----
            wqkv_sb = cpool.tile([128, 4, 3 * E], F16)
            for e in range(4):
                nc.sync.dma_start(wqkv_sb[:, e, :],
                                  wb_d[:, WOFF + 3 * E * e:
                                       WOFF + 3 * E * (e + 1)])
            wp_sb = cpool.tile([128, 4, E], F16)
            for e in range(4):
                nc.sync.dma_start(wp_sb[:, e, :],
                                  wb_d[:, POFF + E * e:POFF + E * (e + 1)])
            mask_sb = cpool.tile([128, sum(KTS)], F16)
            nc.sync.dma_start(mask_sb[:], wb_d[:, MOFF:MOFF + sum(KTS)])
            ident = cpool.tile([128, 128], F16)
            nc.sync.dma_start(ident[:], wb_d[:, IOFF:IOFF + 128])
            xs_sb = cpool.tile([1, T_pad], F16)
            nc.sync.dma_start(xs_sb[:], xs_d[:, :])
            ones_sb = cpool.tile([128, 128], F16)
            nc.vector.memset(ones_sb[:], 1.0)

            def proj_row(xT, r, qb0, w):
                """qkT row-tile r for q-block at qb0 (reads SBUF x^T)."""
                ps = mpsum.tile([128, 512], F32, tag="mp", name="qkps")
                for e in range(4):
                    nc.tensor.matmul(
                        ps[:, :w],
                        wqkv_sb[:, e, 128 * r:128 * (r + 1)],
                        xT[:, e, qb0:qb0 + w],
                        start=(e == 0), stop=(e == 3))
                return ps

            for _rep in range(reps):
              for g in range(len(lengths)):
                n_pad = lengths[g]
                KT = KTS[g]
                QBS = _qb_splits(n_pad)

                # ---- stage A: load + transpose + dequant x ----
                xT = xtpool.tile([128, 4, L0], F16, tag="xT", name="xT")
                qb0 = 0
                for w in QBS:
                    c0 = SOFFS[g] + qb0
                    bc_ps = mpsum.tile([128, 512], F32, tag="mp", name="bcx")
                    nc.tensor.matmul(bc_ps[:, :w], ones_sb[0:1, :],
                                     xs_sb[0:1, c0:c0 + w],
                                     start=True, stop=True)
                    bc_sb = smallpool.tile([128, 512], F16, tag="bcs",
                                           name="bcxs")
                    nc.vector.tensor_copy(bc_sb[:, :w], bc_ps[:, :w])
                    for tl in range((w + 127) // 128):
                        tw = min(128, w - 128 * tl)
                        r0 = c0 + 128 * tl
                        xq_sb = xinpool.tile([128, E], I8, tag="xq8",
                                             name="xq8")
                        nc.sync.dma_start(xq_sb[:tw, :], xq_d[r0:r0 + tw, :])
                        xn = xinpool.tile([128, E], F16, tag="xn", name="xn")
                        nc.vector.tensor_copy(xn[:tw, :], xq_sb[:tw, :])
                        tp = mpsum.tile([128, 4, 128], F16, tag="mp",
                                        name="tpps")
                        for e in range(4):
                            nc.tensor.transpose(
                                tp[:, e, :tw],
                                xn[:tw, 128 * e:128 * (e + 1)],
                                ident[:tw, :tw])
                        for e in range(4):
                            nc.vector.tensor_mul(
                                xT[:, e, qb0 + 128 * tl:qb0 + 128 * tl + tw],
                                tp[:, e, :tw],
                                bc_sb[:, 128 * tl:128 * tl + tw])
                    qb0 += w

                # ---- stage B: QKV projection from SBUF x^T ----
                qT_sb = qkvpool.tile([128, 4, L0], F16, tag="qT", name="qT")
                kT_sb = qkvpool.tile([128, 4, L0], F16, tag="kT", name="kT")
                v_sb = qkvpool.tile([128, KT0, E], F16, tag="v", name="v")
                qb0 = 0
                for w in QBS:
                    for r in range(4):
                        ps = proj_row(xT, r, qb0, w)
                        nc.vector.tensor_copy(qT_sb[:, r, qb0:qb0 + w],
                                              ps[:, :w])
                    for r in range(4, 8):
                        ps = proj_row(xT, r, qb0, w)
                        nc.vector.tensor_copy(kT_sb[:, r - 4, qb0:qb0 + w],
                                              ps[:, :w])
                    for tl in range((w + 127) // 128):
                        tt = (qb0 + 128 * tl) // 128
                        tw = min(128, w - 128 * tl)
                        ps = mpsum.tile([128, 512], F32, tag="mp", name="vps")
                        for e in range(4):
                            nc.tensor.matmul(
                                ps[:tw, :],
                                xT[:, e, qb0 + 128 * tl:qb0 + 128 * tl + tw],
                                wqkv_sb[:, e, 2 * E:3 * E],
                                start=(e == 0), stop=(e == 3))
                        nc.vector.tensor_copy(v_sb[:tw, tt, :], ps[:tw, :])
                    qb0 += w

                # ---- stage C: attention + out-projection + quantize ----
                qb0 = 0
                for w in QBS:
                    ctxn = ctxnpool.tile([128, 4, 512], F16, tag="ctxn",
                                         name="ctxn")
                    for quad in range(2):
                        ctx_ps = [cpsum.tile([128, 512], F32, tag="cp",
                                             name=f"ctxps{p}")
                                  for p in range(2)]
                        den_ps = mpsum.tile([128, 512], F32, tag="mp",
                                            name="denps")
                        for kt in range(KT):
                            tkw = min(128, n_pad - 128 * kt)
                            for pr in range(2):
                                rt = 2 * quad + pr
                                s_ps = spsum.tile([128, 2, 512], F32,
                                                  tag="sp", name="sps")
                                for j in range(2):
                                    po = 64 * j
                                    nc.tensor.matmul(
                                        s_ps[:tkw, j, :w],
                                        kT_sb[po:po + 64, rt,
                                              128 * kt:128 * kt + tkw],
                                        qT_sb[po:po + 64, rt, qb0:qb0 + w],
                                        start=True, stop=True,
                                        tile_position=(po, 0))
                                pt = ptpool.tile([128, 2, 512], F16,
                                                 tag="pt", name="pt")
                                nc.scalar.activation(
                                    pt[:tkw, :, :w], s_ps[:tkw, :, :w],
                                    mybir.ActivationFunctionType.Exp,
                                    bias=mask_sb[:tkw, MOFFS[g] + kt:
                                                 MOFFS[g] + kt + 1],
                                    scale=0.125)
                                for j in range(2):
                                    h = 4 * quad + 2 * pr + j
                                    i = 2 * pr + j
                                    nc.tensor.matmul(
                                        ctx_ps[pr][64 * j:64 * (j + 1), :w],
                                        v_sb[:tkw, kt, 64 * h:64 * (h + 1)],
                                        pt[:tkw, j, :w],
                                        start=(kt == 0),
                                        stop=(kt == KT - 1),
                                        tile_position=(0, 64 * j))
                                    nc.tensor.matmul(
                                        den_ps[32 * i:32 * i + 1, :w],
                                        ones_sb[:tkw, 0:1],
                                        pt[:tkw, j, :w],
                                        start=(kt == 0),
                                        stop=(kt == KT - 1),
                                        tile_position=(0, 32 * i))
                        rdenr = smallpool.tile([128, 512], F16,
                                               tag="rdenr", name="rdenr")
                        with nc.allow_low_precision(reason="f32r rounding"):
                            for i in range(4):
                                nc.vector.reciprocal(
                                    rdenr[32 * i:32 * i + 1, :w],
                                    den_ps[32 * i:32 * i + 1, :w])
                        for p in range(2):
                            bc_ps = mpsum.tile([128, 512], F32, tag="mp",
                                               name="bcps")
                            for j in range(2):
                                i = 2 * p + j
                                nc.tensor.matmul(
                                    bc_ps[64 * j:64 * (j + 1), :w],
                                    ones_sb[32 * i:32 * i + 1, 0:64],
                                    rdenr[32 * i:32 * i + 1, :w],
                                    start=True, stop=True,
                                    tile_position=(32 * i, 64 * j))
                            bc_sb = smallpool.tile([128, 512], F32,
                                                   tag="bcs2", name="bcsb")
                            nc.vector.tensor_copy(bc_sb[:, :w], bc_ps[:, :w])
                            nc.vector.tensor_mul(
                                ctxn[:, 2 * quad + p, :w],
                                ctx_ps[p][:, :w], bc_sb[:, :w])
                    # ---- fused out projection + int8 quantize ----
                    tl0 = 0
                    while tl0 < w:
                        ts_ = min(128, w - tl0)
                        yps = mpsum.tile([128, 512], F32, tag="mp",
                                         name="yps")
                        for e in range(4):
                            nc.tensor.matmul(
                                yps[:ts_, :],
                                ctxn[:, e, tl0:tl0 + ts_],
                                wp_sb[:, e, :],
                                start=(e == 0), stop=(e == 3))
                        m_sb = smallpool.tile([128, 1], F32, tag="ym",
                                              name="ym")
                        nc.vector.tensor_reduce(
                            m_sb[:ts_, :], yps[:ts_, :],
                            mybir.AxisListType.XYZW, mybir.AluOpType.max,
                            apply_absolute_value=True)
                        nc.vector.tensor_scalar_mul(m_sb[:ts_, :],
                                                    m_sb[:ts_, :],
                                                    1.0 / 127.0)
                        nc.vector.tensor_scalar_max(m_sb[:ts_, :],
                                                    m_sb[:ts_, :], 1e-20)
                        r_sb = smallpool.tile([128, 1], F32, tag="yr",
                                              name="yr")
                        with nc.allow_low_precision(reason="quant scale"):
                            nc.vector.reciprocal(r_sb[:ts_, :], m_sb[:ts_, :])
                        yq_sb = ypool.tile([128, 512], I8, tag="yq",
                                           name="yqsb")
                        nc.scalar.activation(
                            yq_sb[:ts_, :], yps[:ts_, :],
                            mybir.ActivationFunctionType.Copy,
                            scale=r_sb[:ts_, 0:1])
                        ys_sb = ypool.tile([128, 1], F16, tag="ys",
                                           name="yssb")
                        nc.vector.tensor_copy(ys_sb[:ts_, :], m_sb[:ts_, :])
                        row0 = SOFFS[g] + qb0 + tl0
                        nc.sync.dma_start(yq_d[row0:row0 + ts_, :],
                                          yq_sb[:ts_, :])
                        nc.sync.dma_start(ys_d[row0:row0 + ts_, :],
                                          ys_sb[:ts_, :])
                        tl0 += ts_
                    qb0 += w

    nc.compile()
    return nc


def _get_runner(lengths):
    """Compile (once) and return the cached jitted runner + state dict."""
    key = tuple(lengths)
    if key in _cache:
        return _cache[key]

    nc = _build(key)
    bass2jax.install_neuronx_cc_hook()
    partition_name = (nc.partition_id_tensor.name
                      if nc.partition_id_tensor else None)
    in_names, out_names, out_avals = [], [], []
    for alloc in nc.m.functions[0].allocations:
        if not isinstance(alloc, mybir.MemoryLocationSet):
            continue
        name = alloc.memorylocations[0].name
        if alloc.kind == "ExternalInput":
            if name != partition_name:
                in_names.append(name)
        elif alloc.kind == "ExternalOutput":
            out_names.append(name)
            out_avals.append(jax.core.ShapedArray(
                tuple(alloc.tensor_shape), mybir.dt.np(alloc.dtype)))
    in_full = list(in_names)        # no donated zero outputs
    if partition_name is not None:
        in_full.append(partition_name)

    def _body(*args):
        operands = list(args)
        if partition_name is not None:
            operands.append(bass2jax.partition_id_tensor())
        outs = bass2jax._bass_exec_p.bind(
            *operands,
            out_avals=tuple(out_avals),
            in_names=tuple(in_full),
            out_names=tuple(out_names),
            lowering_input_output_aliases=(),
            sim_require_finite=True,
            sim_require_nnan=True,
            nc=nc,
        )
        return tuple(outs)

    devices = jax.devices()[:N_CORES]
    mesh = Mesh(np.asarray(devices), ("core",))
    spec = PartitionSpec("core")
    if N_CORES == 1:
        jfn = jax.jit(_body, keep_unused=True)
        sharding = jax.sharding.SingleDeviceSharding(devices[0])
    else:
        jfn = jax.jit(
            shard_map(_body, mesh=mesh,
                      in_specs=(spec,) * len(in_names),
                      out_specs=(spec,) * len(out_names),
                      check_rep=False),
            keep_unused=True,
        )
        sharding = NamedSharding(mesh, spec)
    # compile AOT with effects suppressed -> C++ fast-path dispatch
    sample = []
    for alloc in nc.m.functions[0].allocations:
        if not isinstance(alloc, mybir.MemoryLocationSet):
            continue
        if alloc.kind == "ExternalInput" and                 alloc.memorylocations[0].name in in_names:
            shape = list(alloc.tensor_shape)
            shape[0] *= N_CORES
            sample.append(jax.ShapeDtypeStruct(
                tuple(shape), mybir.dt.np(alloc.dtype), sharding=sharding))
    try:
        fn = bass2jax.fast_dispatch_compile(
            lambda: jfn.lower(*sample).compile())
    except Exception:
        fn = jfn
    state = {
        "fn": fn,
        "in_names": in_names,
        "out_names": out_names,
        "sharding": sharding,
        "wb_dev": None,    # list per wave
        "w_raw": None,
    }
    _cache[key] = state
    return state


def _pack_weights(state, lengths, counts, assign, in_proj_w, out_proj_w,
                  lin_w):
    """Per-wave weight blobs (weights identical; masks differ per lane)."""
    T_pad, SOFFS, KTS, MOFFS, WOFF, POFF, MOFF, IOFF, WCOL = _layout(lengths)
    wqkvT = np.ascontiguousarray(in_proj_w.T).astype(np.float16)
    wpT = np.ascontiguousarray(out_proj_w.T @ lin_w.T).astype(np.float16)
    base = np.zeros((128, WCOL), np.float16)
    for e in range(4):
        base[:, WOFF + 3 * E * e:WOFF + 3 * E * (e + 1)] = \
            wqkvT[128 * e:128 * (e + 1), :]
        base[:, POFF + E * e:POFF + E * (e + 1)] = \
            wpT[128 * e:128 * (e + 1), :]
    base[:, MOFF:IOFF] = NEG
    base[:, IOFF:IOFF + 128] = np.eye(128, dtype=np.float16)

    wb_dev = []
    for v in range(WAVES):
        wbs = []
        for c in range(N_CORES):
            lane = v * N_CORES + c
            wb = base.copy()
            for s in range(GPL):
                g = assign[lane][s]
                n = int(counts[g])
                for kt in range(KTS[s]):
                    valid = min(max(n - 128 * kt, 0), 128)
                    wb[:valid, MOFF + MOFFS[s] + kt] = -8.0
            wbs.append(wb)
        cat = np.concatenate(wbs, axis=0)
        arr = jax.device_put(cat, state["sharding"])
        wb_dev.append(arr)
    jax.block_until_ready(wb_dev)
    state["wb_dev"] = wb_dev


def kernel(x, batch, in_proj_w, in_proj_b, out_proj_w, out_proj_b,
           lin_w, lin_b):
    x = np.ascontiguousarray(np.asarray(x, dtype=np.float32))
    b = np.asarray(batch).astype(np.int64)
    in_proj_w = np.asarray(in_proj_w, dtype=np.float32)
    in_proj_b = np.asarray(in_proj_b, dtype=np.float32)
    out_proj_w = np.asarray(out_proj_w, dtype=np.float32)
    out_proj_b = np.asarray(out_proj_b, dtype=np.float32)
    lin_w = np.asarray(lin_w, dtype=np.float32)
    lin_b = np.asarray(lin_b, dtype=np.float32)

    assert not in_proj_b.any() and not out_proj_b.any() \
        and not lin_b.any(), "nonzero biases not supported by this build"

    T = x.shape[0]
    counts = np.bincount(b, minlength=NG)
    assert counts.sum() == T and len(counts) == NG
    offsets = np.concatenate([[0], np.cumsum(counts)[:-1]])
    order = np.argsort(-counts, kind="stable")
    # lane l, slot s holds graph order[LANES*s + l]; slot padded to the
    # rank-(LANES*s) length so every lane shares one layout/NEFF.
    assign = [[int(order[LANES * s + l]) for s in range(GPL)]
              for l in range(LANES)]
    lengths = tuple(int(counts[order[LANES * s]]) for s in range(GPL))

    state = _get_runner(lengths)
    T_pad, SOFFS, *_ = _layout(lengths)

    w_raw = (in_proj_w, out_proj_w, lin_w)
    if state["w_raw"] is None or not all(
            np.array_equal(a, c) for a, c in zip(w_raw, state["w_raw"])):
        _pack_weights(state, lengths, counts, assign, in_proj_w,
                      out_proj_w, lin_w)
        state["w_raw"] = tuple(a.copy() for a in w_raw)

    name_idx = {n: i for i, n in enumerate(state["in_names"])}
    fn = state["fn"]
    out = np.empty((T, E), np.float32)
    tmp = np.empty((max(lengths), E), np.float32)

    outs = [None] * WAVES
    threads = []
    oidx = {n: i for i, n in enumerate(state["out_names"])}

    def _fetch(v):
        fetched = [np.asarray(o) for o in outs[v]]
        yq = fetched[oidx["yq"]].reshape(N_CORES, T_pad, E)
        ys = fetched[oidx["ys"]].reshape(N_CORES, T_pad, 1)
        for c in range(N_CORES):
            lane = v * N_CORES + c
            for s in range(GPL):
                g = assign[lane][s]
                n = int(counts[g])
                o = int(offsets[g])
                r0 = SOFFS[s]
                np.multiply(yq[c, r0:r0 + n], ys[c, r0:r0 + n],
                            out=out[o:o + n], casting="unsafe")

    for v in range(WAVES):
        # ---- quantize + pack this wave's graphs ----
        xq = np.zeros((N_CORES * T_pad, E), np.int8)
        xs = np.zeros((N_CORES, T_pad), np.float16)
        for c in range(N_CORES):
            lane = v * N_CORES + c
            cb = c * T_pad
            for s in range(GPL):
                g = assign[lane][s]
                n = int(counts[g])
                o = int(offsets[g])
                xg = x[o:o + n]
                m = np.maximum(xg.max(axis=1), -xg.min(axis=1))
                s16 = (m * (1.0 / 127.0)).astype(np.float16)
                s32 = s16.astype(np.float32)
                s32[s32 == 0] = 1.0
                t = tmp[:n]
                np.multiply(xg, (1.0 / s32)[:, None], out=t)
                np.rint(t, out=t)
                xq[cb + SOFFS[s]:cb + SOFFS[s] + n] = t
                xs[c, SOFFS[s]:SOFFS[s] + n] = s16
        args = [None] * len(state["in_names"])
        args[name_idx["xq"]] = xq
        args[name_idx["xs"]] = xs
        args[name_idx["wb"]] = state["wb_dev"][v]
        outs[v] = fn(*args)
        th = threading.Thread(target=_fetch, args=(v,))
        th.start()
        threads.append(th)

    for th in threads:
        th.join()
    return out


# revision 13
# speedup vs baseline: 1.0456x; 1.0456x over previous
"""Trainium2 Bass kernel for CrossGraphAttention (ragged per-graph MHA + linear).

The grading cost is dominated by host<->device transfer through the axon
PJRT relay (~25-130 MB/s, drifting; device compute is ~1.5 ms/call), so the
design minimizes bytes on the wire per call and overlaps transfer
directions:

  * int8 per-row quantized I/O: x is quantized host-side to int8 with a
    per-token fp16 scale (rel-err contribution ~8e-3, tolerance 2e-2); y is
    quantized device-side to int8 with a per-token fp16 scale (the HW
    converts float->int8 with round-to-nearest; validated by probe). 8.4 MB
    up + 8.4 MB down total instead of 2x 16.8 MB fp16 and 17 MB of donated
    zero-output upload in the stock path (52 MB -> 17 MB per call; measured
    4.1x faster than the previous kernel in an interleaved A/B).
  * weights/masks/identity live in small per-wave blobs uploaded ONCE and
    cached as committed device arrays across kernel() calls; the jit
    executable and NEFF are cached too, so warm calls skip retrace.
  * custom mirror of bass2jax.run_bass_via_pjrt without output-buffer
    donation: PJRT-allocated uninitialized outputs are fine because the
    host only reads rows the kernel wrote.
  * the call is split into WAVES pipelined executions of one shared NEFF
    (graphs are grouped into N_CORES*WAVES lanes; slot s of every lane is
    padded to the length of the rank-(LANES*s) graph so all lanes share one
    layout). Wave k+1's host quantize + upload overlap wave k's execute +
    download (the relay carries H2D and D2H concurrently at ~1.5x the
    serial rate); fetches run in background threads.
  * x ships in natural [token, 512] int8 layout (host does one vectorized
    quantize pass + contiguous row copies); the device converts to fp16,
    transposes via PE identity matmuls, and de-quantizes columns with a
    rank-1 broadcast of the scale row, producing the x^T layout the QKV
    projection needs. No strided host-side transposes in the per-call path.

Final config: 1 core x 4 waves (the relay is a single ~40 MB/s shared
pipe under current conditions -- extra cores only add dispatch overhead;
4 waves keep upload/execute/download fully overlapped). Measured: device
exec 1.15 ms/wave (reps-marginal), ~82 ms zero-payload dispatch RTT per
call (fully hidden by the wave pipeline), warm call within ~2% of the
bytes/bandwidth model -- the kernel is transport-floor-bound.

Device program per core/wave (data-parallel over graphs):
  1. load+transpose+dequant x into SBUF-resident x^T (per graph).
  2. QKV projection from SBUF x^T (q^T, k^T per head row-tiles; V natural).
  3. attention: scores TRANSPOSED (S^T[k,q]) per head-pair, exp fused with
     PSUM->SBUF eviction (scalar engine) with key-padding mask via a
     per-partition bias (-60000 -> exp 0; valid tokens get -8 so fp16 P
     stays in range, cancels in softmax); denominator via ones-matmul;
     ctx^T accumulated over k-tiles in PSUM; normalization by 1/denom via
     rank-1 broadcast matmul + vector multiply.
  4. fused output projection y = ctx @ (lin_w @ out_proj_w)^T, then per-row
     absmax -> int8 quantize + fp16 scale, DMA out.
"""

import threading

import numpy as np

import concourse.mybir as mybir
import concourse.tile as tile
from concourse import bacc, bass2jax

import jax
from jax.sharding import Mesh, PartitionSpec, NamedSharding
from jax.experimental.shard_map import shard_map

F32 = mybir.dt.float32
F16 = mybir.dt.float16
I8 = mybir.dt.int8

NG = 16          # number of graphs
N_CORES = 1      # cores used per wave (relay is one ~40MB/s pipe; more cores
                 # only add dispatch overhead)
WAVES = 4        # pipelined executions per call
LANES = N_CORES * WAVES
GPL = NG // LANES     # graphs (slots) per lane
E = 512
H = 8
D = 64
NEG = -60000.0   # exp(scale*s + NEG) == 0; representable in fp16

_cache = {}


def _qb_splits(n):
    out = [512] * (n // 512)
    if n % 512:
        out.append(n % 512)
    return out


def _layout(lengths):
    """lengths: per-slot padded graph lengths (shared by all lanes).
    Returns slot offsets + layout of the per-lane weight blob [128, WCOL]."""
    T_pad = sum(lengths)
    soffs = [0]
    for L in lengths[:-1]:
        soffs.append(soffs[-1] + L)
    kts = [(L + 127) // 128 for L in lengths]
    moffs = [0]
    for k in kts[:-1]:
        moffs.append(moffs[-1] + k)
    woff = 0                       # 4 chunks of 1536 cols (rows 128e, Wqkv^T)
    poff = woff + 4 * 3 * E        # 4 chunks of 512 cols (rows 128e, Wp^T)
    moff = poff + 4 * E            # sum(kts) cols of per-partition mask bias
    ioff = moff + sum(kts)         # 128 cols fp16 identity
    wcol = ioff + 128
    return T_pad, soffs, kts, moffs, woff, poff, moff, ioff, wcol


def _build(lengths, reps=1):
    """Build + compile the Bass program for per-slot graph lengths."""
    T_pad, SOFFS, KTS, MOFFS, WOFF, POFF, MOFF, IOFF, WCOL = _layout(lengths)
    L0 = max(lengths)
    KT0 = (L0 + 127) // 128

    nc = bacc.Bacc("TRN2", target_bir_lowering=False, debug=False,
                   enable_asserts=False)

    xq_d = nc.dram_tensor("xq", [T_pad, E], I8, kind="ExternalInput")
    xs_d = nc.dram_tensor("xs", [1, T_pad], F16, kind="ExternalInput")
    wb_d = nc.dram_tensor("wb", [128, WCOL], F16, kind="ExternalInput")
    yq_d = nc.dram_tensor("yq", [T_pad, E], I8, kind="ExternalOutput")
    ys_d = nc.dram_tensor("ys", [T_pad, 1], F16, kind="ExternalOutput")

    with tile.TileContext(nc) as tc:
        with (
            tc.tile_pool(name="const", bufs=1) as cpool,
            tc.tile_pool(name="xt", bufs=2) as xtpool,
            tc.tile_pool(name="xin", bufs=3) as xinpool,
            tc.tile_pool(name="qkv", bufs=2) as qkvpool,
            tc.tile_pool(name="pt", bufs=4) as ptpool,
            tc.tile_pool(name="small", bufs=3) as smallpool,
            tc.tile_pool(name="ctxn", bufs=3) as ctxnpool,
            tc.tile_pool(name="yout", bufs=3) as ypool,
            tc.tile_pool(name="spsum", bufs=2, space="PSUM") as spsum,
            tc.tile_pool(name="cpsum", bufs=2, space="PSUM") as cpsum,
            tc.tile_pool(name="mpsum", bufs=2, space="PSUM") as mpsum,
        ):
            # ---- constants / weights (resident) ----
            wqkv_sb = cpool.tile([128, 4, 3 * E], F16)
            for e in range(4):
                nc.sync.dma_start(wqkv_sb[:, e, :],
                                  wb_d[:, WOFF + 3 * E * e:
                                       WOFF + 3 * E * (e + 1)])
            wp_sb = cpool.tile([128, 4, E], F16)
            for e in range(4):
                nc.sync.dma_start(wp_sb[:, e, :],
                                  wb_d[:, POFF + E * e:POFF + E * (e + 1)])
            mask_sb = cpool.tile([128, sum(KTS)], F16)
            nc.sync.dma_start(mask_sb[:], wb_d[:, MOFF:MOFF + sum(KTS)])
            ident = cpool.tile([128, 128], F16)
            nc.sync.dma_start(ident[:], wb_d[:, IOFF:IOFF + 128])
            xs_sb = cpool.tile([1, T_pad], F16)
            nc.sync.dma_start(xs_sb[:], xs_d[:, :])
            ones_sb = cpool.tile([128, 128], F16)
            nc.vector.memset(ones_sb[:], 1.0)

            def proj_row(xT, r, qb0, w):
                """qkT row-tile r for q-block at qb0 (reads SBUF x^T)."""
                ps = mpsum.tile([128, 512], F32, tag="mp", name="qkps")
                for e in range(4):
                    nc.tensor.matmul(
                        ps[:, :w],
                        wqkv_sb[:, e, 128 * r:128 * (r + 1)],
                        xT[:, e, qb0:qb0 + w],
                        start=(e == 0), stop=(e == 3))
                return ps

            for _rep in range(reps):
              for g in range(len(lengths)):
                n_pad = lengths[g]
                KT = KTS[g]
                QBS = _qb_splits(n_pad)

                # ---- stage A: load + transpose + dequant x ----
                xT = xtpool.tile([128, 4, L0], F16, tag="xT", name="xT")
                qb0 = 0
                for w in QBS:
                    c0 = SOFFS[g] + qb0
                    bc_ps = mpsum.tile([128, 512], F32, tag="mp", name="bcx")
                    nc.tensor.matmul(bc_ps[:, :w], ones_sb[0:1, :],
                                     xs_sb[0:1, c0:c0 + w],
                                     start=True, stop=True)
                    bc_sb = smallpool.tile([128, 512], F16, tag="bcs",
                                           name="bcxs")
                    nc.vector.tensor_copy(bc_sb[:, :w], bc_ps[:, :w])
                    for tl in range((w + 127) // 128):
                        tw = min(128, w - 128 * tl)
                        r0 = c0 + 128 * tl
                        xq_sb = xinpool.tile([128, E], I8, tag="xq8",
                                             name="xq8")
                        nc.sync.dma_start(xq_sb[:tw, :], xq_d[r0:r0 + tw, :])
                        xn = xinpool.tile([128, E], F16, tag="xn", name="xn")
                        nc.vector.tensor_copy(xn[:tw, :], xq_sb[:tw, :])
                        tp = mpsum.tile([128, 4, 128], F16, tag="mp",
                                        name="tpps")
                        for e in range(4):
                            nc.tensor.transpose(
                                tp[:, e, :tw],
                                xn[:tw, 128 * e:128 * (e + 1)],
                                ident[:tw, :tw])
                        for e in range(4):
                            nc.vector.tensor_mul(
                                xT[:, e, qb0 + 128 * tl:qb0 + 128 * tl + tw],
                                tp[:, e, :tw],
                                bc_sb[:, 128 * tl:128 * tl + tw])
                    qb0 += w

                # ---- stage B: QKV projection from SBUF x^T ----
                qT_sb = qkvpool.tile([128, 4, L0], F16, tag="qT", name="qT")
                kT_sb = qkvpool.tile([128, 4, L0], F16, tag="kT", name="kT")
                v_sb = qkvpool.tile([128, KT0, E], F16, tag="v", name="v")
                qb0 = 0
                for w in QBS:
                    for r in range(4):
                        ps = proj_row(xT, r, qb0, w)
                        nc.vector.tensor_copy(qT_sb[:, r, qb0:qb0 + w],
                                              ps[:, :w])
                    for r in range(4, 8):
                        ps = proj_row(xT, r, qb0, w)
                        nc.vector.tensor_copy(kT_sb[:, r - 4, qb0:qb0 + w],
                                              ps[:, :w])
                    for tl in range((w + 127) // 128):
                        tt = (qb0 + 128 * tl) // 128
                        tw = min(128, w - 128 * tl)
                        ps = mpsum.tile([128, 512], F32, tag="mp", name="vps")
                        for e in range(4):
                            nc.tensor.matmul(
                                ps[:tw, :],
                                xT[:, e, qb0 + 128 * tl:qb0 + 128 * tl + tw],
                                wqkv_sb[:, e, 2 * E:3 * E],
                                start=(e == 0), stop=(e == 3))
                        nc.vector.tensor_copy(v_sb[:tw, tt, :], ps[:tw, :])
                    qb0 += w

                # ---- stage C: attention + out-projection + quantize ----
                qb0 = 0
                for w in QBS:
                    ctxn = ctxnpool.tile([128, 4, 512], F16, tag="ctxn",
                                         name="ctxn")
                    for quad in range(2):
                        ctx_ps = [cpsum.tile([128, 512], F32, tag="cp",
                                             name=f"ctxps{p}")
                                  for p in range(2)]
                        den_ps = mpsum.tile([128, 512], F32, tag="mp",
                                            name="denps")
                        for kt in range(KT):
                            tkw = min(128, n_pad - 128 * kt)
                            for pr in range(2):
                                rt = 2 * quad + pr
                                s_ps = spsum.tile([128, 2, 512], F32,
                                                  tag="sp", name="sps")
                                for j in range(2):
                                    po = 64 * j
                                    nc.tensor.matmul(
                                        s_ps[:tkw, j, :w],
                                        kT_sb[po:po + 64, rt,
                                              128 * kt:128 * kt + tkw],
                                        qT_sb[po:po + 64, rt, qb0:qb0 + w],
                                        start=True, stop=True,
                                        tile_position=(po, 0))
                                pt = ptpool.tile([128, 2, 512], F16,
                                                 tag="pt", name="pt")
                                nc.scalar.activation(
                                    pt[:tkw, :, :w], s_ps[:tkw, :, :w],
                                    mybir.ActivationFunctionType.Exp,
                                    bias=mask_sb[:tkw, MOFFS[g] + kt:
                                                 MOFFS[g] + kt + 1],
                                    scale=0.125)
                                for j in range(2):
                                    h = 4 * quad + 2 * pr + j
                                    i = 2 * pr + j
                                    nc.tensor.matmul(
                                        ctx_ps[pr][64 * j:64 * (j + 1), :w],
                                        v_sb[:tkw, kt, 64 * h:64 * (h + 1)],
                                        pt[:tkw, j, :w],
                                        start=(kt == 0),
                                        stop=(kt == KT - 1),
                                        tile_position=(0, 64 * j))
                                    nc.tensor.matmul(
                                        den_ps[32 * i:32 * i + 1, :w],
                                        ones_sb[:tkw, 0:1],
                                        pt[:tkw, j, :w],
                                        start=(kt == 0),
                                        stop=(kt == KT - 1),
                                        tile_position=(0, 32 * i))
                        rdenr = smallpool.tile([128, 512], F16,
                                               tag="rdenr", name="rdenr")
                        with nc.allow_low_precision(reason="f32r rounding"):
                            for i in range(4):
                                nc.vector.reciprocal(
                                    rdenr[32 * i:32 * i + 1, :w],
                                    den_ps[32 * i:32 * i + 1, :w])
                        for p in range(2):
                            bc_ps = mpsum.tile([128, 512], F32, tag="mp",
                                               name="bcps")
                            for j in range(2):
                                i = 2 * p + j
                                nc.tensor.matmul(
                                    bc_ps[64 * j:64 * (j + 1), :w],
                                    ones_sb[32 * i:32 * i + 1, 0:64],
                                    rdenr[32 * i:32 * i + 1, :w],
                                    start=True, stop=True,
                                    tile_position=(32 * i, 64 * j))
                            bc_sb = smallpool.tile([128, 512], F32,
                                                   tag="bcs2", name="bcsb")
                            nc.vector.tensor_copy(bc_sb[:, :w], bc_ps[:, :w])
                            nc.vector.tensor_mul(
                                ctxn[:, 2 * quad + p, :w],
                                ctx_ps[p][:, :w], bc_sb[:, :w])
                    # ---- fused out projection + int8 quantize ----
                    tl0 = 0
                    while tl0 < w:
                        ts_ = min(128, w - tl0)
                        yps = mpsum.tile([128, 512], F32, tag="mp",
                                         name="yps")
                        for e in range(4):
                            nc.tensor.matmul(
                                yps[:ts_, :],
                                ctxn[:, e, tl0:tl0 + ts_],
                                wp_sb[:, e, :],
                                start=(e == 0), stop=(e == 3))
                        m_sb = smallpool.tile([128, 1], F32, tag="ym",
                                              name="ym")
                        nc.vector.tensor_reduce(
                            m_sb[:ts_, :], yps[:ts_, :],
                            mybir.AxisListType.XYZW, mybir.AluOpType.max,
                            apply_absolute_value=True)
                        nc.vector.tensor_scalar_mul(m_sb[:ts_, :],
                                                    m_sb[:ts_, :],
                                                    1.0 / 127.0)
                        nc.vector.tensor_scalar_max(m_sb[:ts_, :],
                                                    m_sb[:ts_, :], 1e-20)
                        r_sb = smallpool.tile([128, 1], F32, tag="yr",
                                              name="yr")
                        with nc.allow_low_precision(reason="quant scale"):
                            nc.vector.reciprocal(r_sb[:ts_, :], m_sb[:ts_, :])
                        yq_sb = ypool.tile([128, 512], I8, tag="yq",
                                           name="yqsb")
                        nc.scalar.activation(
                            yq_sb[:ts_, :], yps[:ts_, :],
                            mybir.ActivationFunctionType.Copy,
                            scale=r_sb[:ts_, 0:1])
                        ys_sb = ypool.tile([128, 1], F16, tag="ys",
                                           name="yssb")
                        nc.vector.tensor_copy(ys_sb[:ts_, :], m_sb[:ts_, :])
                        row0 = SOFFS[g] + qb0 + tl0
                        nc.sync.dma_start(yq_d[row0:row0 + ts_, :],
                                          yq_sb[:ts_, :])
                        nc.sync.dma_start(ys_d[row0:row0 + ts_, :],
                                          ys_sb[:ts_, :])
                        tl0 += ts_
                    qb0 += w

    nc.compile()
    return nc


def _get_runner(lengths):
    """Compile (once) and return the cached jitted runner + state dict."""
    key = tuple(lengths)
    if key in _cache:
        return _cache[key]

    nc = _build(key)
    bass2jax.install_neuronx_cc_hook()
    partition_name = (nc.partition_id_tensor.name
                      if nc.partition_id_tensor else None)
    in_names, out_names, out_avals = [], [], []
    for alloc in nc.m.functions[0].allocations:
        if not isinstance(alloc, mybir.MemoryLocationSet):
            continue
        name = alloc.memorylocations[0].name
        if alloc.kind == "ExternalInput":
            if name != partition_name:
                in_names.append(name)
        elif alloc.kind == "ExternalOutput":
            out_names.append(name)
            out_avals.append(jax.core.ShapedArray(
                tuple(alloc.tensor_shape), mybir.dt.np(alloc.dtype)))
    in_full = list(in_names)        # no donated zero outputs
    if partition_name is not None:
        in_full.append(partition_name)

    def _body(*args):
        operands = list(args)
        if partition_name is not None:
            operands.append(bass2jax.partition_id_tensor())
        outs = bass2jax._bass_exec_p.bind(
            *operands,
            out_avals=tuple(out_avals),
            in_names=tuple(in_full),
            out_names=tuple(out_names),
            lowering_input_output_aliases=(),
            sim_require_finite=True,
            sim_require_nnan=True,
            nc=nc,
        )
        return tuple(outs)

    devices = jax.devices()[:N_CORES]
    mesh = Mesh(np.asarray(devices), ("core",))
    spec = PartitionSpec("core")
    if N_CORES == 1:
        jfn = jax.jit(_body, keep_unused=True)
        sharding = jax.sharding.SingleDeviceSharding(devices[0])
    else:
        jfn = jax.jit(
            shard_map(_body, mesh=mesh,
                      in_specs=(spec,) * len(in_names),
                      out_specs=(spec,) * len(out_names),
                      check_rep=False),
            keep_unused=True,
        )
        sharding = NamedSharding(mesh, spec)
    # compile AOT with effects suppressed -> C++ fast-path dispatch
    sample = []
    for alloc in nc.m.functions[0].allocations:
        if not isinstance(alloc, mybir.MemoryLocationSet):
            continue
        if alloc.kind == "ExternalInput" and                 alloc.memorylocations[0].name in in_names:
            shape = list(alloc.tensor_shape)
            shape[0] *= N_CORES
            sample.append(jax.ShapeDtypeStruct(
                tuple(shape), mybir.dt.np(alloc.dtype), sharding=sharding))
    try:
        fn = bass2jax.fast_dispatch_compile(
            lambda: jfn.lower(*sample).compile())
    except Exception:
        fn = jfn
    state = {
        "fn": fn,
        "in_names": in_names,
        "out_names": out_names,
        "sharding": sharding,
        "wb_dev": None,    # list per wave
        "w_raw": None,
    }
    _cache[key] = state
    return state


def _pack_weights(state, lengths, in_proj_w, out_proj_w, lin_w):
    """Weight blob for one lane (exact slot lengths == that lane's graphs)."""
    T_pad, SOFFS, KTS, MOFFS, WOFF, POFF, MOFF, IOFF, WCOL = _layout(lengths)
    wqkvT = np.ascontiguousarray(in_proj_w.T).astype(np.float16)
    wpT = np.ascontiguousarray(out_proj_w.T @ lin_w.T).astype(np.float16)
    wb = np.zeros((128, WCOL), np.float16)
    for e in range(4):
        wb[:, WOFF + 3 * E * e:WOFF + 3 * E * (e + 1)] = \
            wqkvT[128 * e:128 * (e + 1), :]
        wb[:, POFF + E * e:POFF + E * (e + 1)] = \
            wpT[128 * e:128 * (e + 1), :]
    wb[:, MOFF:IOFF] = NEG
    for s, n in enumerate(lengths):
        for kt in range(KTS[s]):
            valid = min(max(n - 128 * kt, 0), 128)
            wb[:valid, MOFF + MOFFS[s] + kt] = -8.0
    wb[:, IOFF:IOFF + 128] = np.eye(128, dtype=np.float16)
    state["wb_dev"] = jax.device_put(wb, state["sharding"])
    jax.block_until_ready(state["wb_dev"])


def kernel(x, batch, in_proj_w, in_proj_b, out_proj_w, out_proj_b,
           lin_w, lin_b):
    x = np.ascontiguousarray(np.asarray(x, dtype=np.float32))
    b = np.asarray(batch).astype(np.int64)
    in_proj_w = np.asarray(in_proj_w, dtype=np.float32)
    in_proj_b = np.asarray(in_proj_b, dtype=np.float32)
    out_proj_w = np.asarray(out_proj_w, dtype=np.float32)
    out_proj_b = np.asarray(out_proj_b, dtype=np.float32)
    lin_w = np.asarray(lin_w, dtype=np.float32)
    lin_b = np.asarray(lin_b, dtype=np.float32)

    assert N_CORES == 1
    assert not in_proj_b.any() and not out_proj_b.any() \
        and not lin_b.any(), "nonzero biases not supported by this build"

    T = x.shape[0]
    counts = np.bincount(b, minlength=NG)
    assert counts.sum() == T and len(counts) == NG
    offsets = np.concatenate([[0], np.cumsum(counts)[:-1]])
    order = np.argsort(-counts, kind="stable")
    # snake assignment over sorted lengths balances per-lane byte totals
    # to within a few rows; each lane gets its EXACT layout (zero padding).
    lanes = [[] for _ in range(WAVES)]
    for i in range(NG):
        r = i % (2 * WAVES)
        lane = r if r < WAVES else 2 * WAVES - 1 - r
        lanes[lane].append(int(order[i]))
    lane_lengths = [tuple(int(counts[g]) for g in lanes[v])
                    for v in range(WAVES)]

    # compile missing lane programs concurrently (cold path only)
    missing = [lt for lt in dict.fromkeys(lane_lengths) if lt not in _cache]
    if len(missing) > 1:
        from concurrent.futures import ThreadPoolExecutor
        with ThreadPoolExecutor(len(missing)) as ex:
            list(ex.map(_get_runner, missing))
    states = [_get_runner(lane_lengths[v]) for v in range(WAVES)]

    w_raw = (in_proj_w, out_proj_w, lin_w)
    wkey = _cache.setdefault("_weights", {"raw": None, "ver": 0})
    if wkey["raw"] is None or not all(
            np.array_equal(a, c) for a, c in zip(w_raw, wkey["raw"])):
        wkey["raw"] = tuple(a.copy() for a in w_raw)
        wkey["ver"] += 1
    for v in range(WAVES):
        st = states[v]
        if st.get("wver") != wkey["ver"]:
            _pack_weights(st, lane_lengths[v], in_proj_w, out_proj_w, lin_w)
            st["wver"] = wkey["ver"]

    out = np.empty((T, E), np.float32)
    tmp = np.empty((int(counts.max()), E), np.float32)
    outs = [None] * WAVES
    threads = []

    def _fetch(v):
        st = states[v]
        T_pad, SOFFS, *_ = _layout(lane_lengths[v])
        oidx = {n: i for i, n in enumerate(st["out_names"])}
        fetched = [np.asarray(o) for o in outs[v]]
        yq = fetched[oidx["yq"]]
        ys = fetched[oidx["ys"]]
        for s, g in enumerate(lanes[v]):
            n = int(counts[g])
            o = int(offsets[g])
            r0 = SOFFS[s]
            np.multiply(yq[r0:r0 + n], ys[r0:r0 + n],
                        out=out[o:o + n], casting="unsafe")

    for v in range(WAVES):
        st = states[v]
        T_pad, SOFFS, *_ = _layout(lane_lengths[v])
        xq = np.zeros((T_pad, E), np.int8)
        xs = np.zeros((1, T_pad), np.float16)
        for s, g in enumerate(lanes[v]):
            n = int(counts[g])
            o = int(offsets[g])
            xg = x[o:o + n]
            m = np.maximum(xg.max(axis=1), -xg.min(axis=1))
            s16 = (m * (1.0 / 127.0)).astype(np.float16)
            s32 = s16.astype(np.float32)
            s32[s32 == 0] = 1.0
            t = tmp[:n]
            np.multiply(xg, (1.0 / s32)[:, None], out=t)
            np.rint(t, out=t)
            xq[SOFFS[s]:SOFFS[s] + n] = t
            xs[0, SOFFS[s]:SOFFS[s] + n] = s16
        name_idx = {n: i for i, n in enumerate(st["in_names"])}
        args = [None] * len(st["in_names"])
        args[name_idx["xq"]] = xq
        args[name_idx["xs"]] = xs
        args[name_idx["wb"]] = st["wb_dev"]
        outs[v] = st["fn"](*args)
        th = threading.Thread(target=_fetch, args=(v,))
        th.start()
        threads.append(th)

    for th in threads:
        th.join()
    return out
